# revision 14
# baseline (speedup 1.0000x reference)
"""Cross-attention kernel for Trainium2, batch-data-parallel over 8 NeuronCores.

Reference computation (per batch element b):
    q = x Wq + bq ; k = c Wk + bk ; v = c Wv + bv          (DIM=1024)
    per head h (16 heads, d=64):
        S = (q_h k_h^T) * d^-0.5 ; P = softmax(S, axis=-1) ; o_h = P v_h
    out = concat_h(o_h) Wo + bo

v2 layout/schedule notes:
  * All matmul contractions sit on SBUF partitions (host passes x^T, c^T).
    Scores are computed transposed, ST=[m,n], so P@V is a plain accumulation
    with stationary V[m,d]; V carries a ones column so the softmax
    denominator falls out of the same matmul.
  * Score matmuls for the two heads of a pair are emitted back-to-back at
    PE row bases 0/64 — the 128x128 array runs both concurrently (~2x).
  * Loop is n-half-outer, head-pair-inner; K/Q projections interleave into
    the first n-half pass and the final projection of n-half 0 overlaps the
    second pass, so TensorE never drains while ACT (exp) catches up.
  * Wk/Wq are repacked per-output-block in DRAM so k_proj(0)/q_proj(0) only
    need a 256KB slice; DMAs are emitted on the critical path order.
  * Softmax tails (1/colsum via exp(-ln)) are batched two blocks at a time
    onto partitions {0,32,64,96} so the 1-partition ACT cost is amortized.
"""

import os

import numpy as np
import ml_dtypes

import concourse.bass as bass
import concourse.bacc as bacc
import concourse.mybir as mybir
import concourse.tile as tile

B = 8
SEQ = 1024          # N == M == 1024
DIM = 1024
H = 16
HD = DIM // H       # 64
SCALE = HD ** -0.5
P = 128
NCH = DIM // P      # 8
HW = HD + 1         # head width in the augmented V (64 values + ones col)

BF16 = mybir.dt.bfloat16
F32 = mybir.dt.float32
NPBF16 = ml_dtypes.bfloat16
EXP = mybir.ActivationFunctionType.Exp
LOG = mybir.ActivationFunctionType.Ln


class _Bacc(bacc.Bacc):
    def insert_act_table_loads(self):
        # Prefer natural_log_exp_and_others (has BOTH Exp and Ln) so the
        # softmax exp and the exp(-ln) reciprocal share one table set —
        # otherwise the pass alternates sets and pays ~2.7us per switch.
        from concourse.hw_specs import get_activation_tables
        import bass_rust as _br
        tables = list(get_activation_tables(self.m.arch).items())
        canon = [k for k, _ in tables]
        tables.sort(key=lambda kv: kv[0] != "natural_log_exp_and_others")
        _br.insert_act_table_loads(self, tables)
        want = canon.index("natural_log_exp_and_others")
        for f in self.m.functions:
            for b in f.blocks:
                for i in b.instructions:
                    if isinstance(i, mybir.InstLoadActFuncSet):
                        i.act_func_set_id = want


def build_nc() -> bass.Bass:
    nc = _Bacc("TRN2")

    xt_d = nc.declare_dram_parameter("xt", [NCH, P, SEQ], BF16, isOutput=False)
    ct_d = nc.declare_dram_parameter("ct", [NCH, P, SEQ], BF16, isOutput=False)
    # wq/wk repacked per output 128-col block: [jq, P(k-part), k-chunk*128]
    wq_d = nc.declare_dram_parameter("wq", [NCH, P, DIM], BF16, isOutput=False)
    wk_d = nc.declare_dram_parameter("wk", [NCH, P, DIM], BF16, isOutput=False)
    # wv/wo stay k-chunk-major: [k, P, DIM]
    wv_d = nc.declare_dram_parameter("wv", [NCH, P, DIM], BF16, isOutput=False)
    wo_d = nc.declare_dram_parameter("wo", [NCH, P, DIM], BF16, isOutput=False)
    bq_d = nc.declare_dram_parameter("bq", [P, NCH], F32, isOutput=False)
    bk_d = nc.declare_dram_parameter("bk", [P, NCH], F32, isOutput=False)
    bv_d = nc.declare_dram_parameter("bv", [DIM], F32, isOutput=False)
    bo_d = nc.declare_dram_parameter("bo", [DIM], F32, isOutput=False)
    out_d = nc.declare_dram_parameter("out", [SEQ, DIM], F32, isOutput=True)

    with tile.TileContext(nc) as tc:
        with (
            tc.tile_pool(name="big", bufs=1) as big,
            tc.tile_pool(name="wts", bufs=1) as wts,
            tc.tile_pool(name="expp", bufs=2) as expp,
            tc.tile_pool(name="csp", bufs=2) as csp,
            tc.tile_pool(name="csp1", bufs=1) as csp1,
            tc.tile_pool(name="otsp", bufs=10) as otsp,
            tc.tile_pool(name="outp", bufs=2) as outp,
            tc.tile_pool(name="ppr", bufs=2, space="PSUM") as ppr,
            tc.tile_pool(name="pot", bufs=2, space="PSUM") as pot,
            tc.tile_pool(name="pst", bufs=2, space="PSUM") as pst,
        ):
            # ---- persistent SBUF tensors ----
            ct_sb = big.tile([P, NCH, SEQ], BF16, tag="ct")
            xt_sb = big.tile([P, NCH, SEQ], BF16, tag="xt")
            # wk/wq layout: [P, jq, k*128]  (lhsT slice = [:, jq, k*128:+128])
            wk_sb = big.tile([P, NCH, DIM], BF16, tag="wk")
            wq_sb = big.tile([P, NCH, DIM], BF16, tag="wq")
            # wv then wo rotate through one slot (wv dead after v_phase)
            wv_sb = wts.tile([P, NCH, DIM], BF16, tag="w", name="wv")
            wo_sb = wts.tile([P, NCH, DIM], BF16, tag="w", name="wo")
            kt_sb = big.tile([P, NCH, SEQ], BF16, tag="kt")
            qt_sb = big.tile([P, NCH, SEQ], BF16, tag="qt")
            ot_sb = big.tile([P, NCH, SEQ], BF16, tag="ot")
            vaug_sb = big.tile([P, NCH, H * HW], BF16, tag="vaug")
            bq_sb = big.tile([P, NCH], F32, tag="bq")
            bk_sb = big.tile([P, NCH], F32, tag="bk")
            bvb_sb = big.tile([P, DIM], F32, tag="bvb")
            bob_sb = big.tile([P, DIM], F32, tag="bob")

            # ---- input DMAs, critical-path order ----
            # k_proj(0, mh) needs wk block 0 plus the mh-half of every ct
            # chunk, so ship ct (and xt) in n-halves: the first matmul can
            # start after ~1.25MB instead of 2.25MB.
            nc.sync.dma_start(out=wk_sb[:, 0, :], in_=wk_d[0])
            for j in range(NCH):
                nc.sync.dma_start(out=ct_sb[:, j, 0:512], in_=ct_d[j][:, 0:512])
            nc.sync.dma_start(out=bq_sb, in_=bq_d[:, :])
            nc.sync.dma_start(out=bk_sb, in_=bk_d[:, :])
            for j in range(NCH):
                nc.sync.dma_start(out=ct_sb[:, j, 512:1024],
                                  in_=ct_d[j][:, 512:1024])
            nc.sync.dma_start(out=wq_sb[:, 0, :], in_=wq_d[0])
            for j in range(NCH):
                nc.sync.dma_start(out=xt_sb[:, j, 0:512], in_=xt_d[j][:, 0:512])
            for j in range(NCH):
                nc.sync.dma_start(out=xt_sb[:, j, 512:1024],
                                  in_=xt_d[j][:, 512:1024])
            for j in range(1, NCH):
                nc.sync.dma_start(out=wk_sb[:, j, :], in_=wk_d[j])
                nc.sync.dma_start(out=wq_sb[:, j, :], in_=wq_d[j])
            for j in range(NCH):
                nc.sync.dma_start(out=wv_sb[:, j, :], in_=wv_d[j])
            for (dst, src) in ((bvb_sb, bv_d), (bob_sb, bo_d)):
                ap = src[:]
                bcast = bass.AP(tensor=ap.tensor, offset=ap.offset,
                                ap=[[0, P]] + ap.ap)
                nc.sync.dma_start(out=dst, in_=bcast)
            # wo reuses wv's slot: DMA waits until v_phase is done with wv.
            for j in range(NCH):
                nc.sync.dma_start(out=wo_sb[:, j, :], in_=wo_d[j])

            vaug4 = vaug_sb.rearrange("p j (h e) -> p j h e", e=HW)
            nc.vector.memset(vaug4[:, :, :, HD:HW], 1.0)

            # ---- projections ----
            def q_proj(jq):
                for mh in range(2):
                    pq = ppr.tile([P, 512], F32, tag="ppr", name="pq")
                    for k in range(NCH):
                        nc.tensor.matmul(
                            pq,
                            lhsT=wq_sb[:, jq, k * P:(k + 1) * P],
                            rhs=xt_sb[:, k, mh * 512:(mh + 1) * 512],
                            start=(k == 0), stop=(k == NCH - 1),
                        )
                    nc.vector.tensor_scalar_add(
                        qt_sb[:, jq, mh * 512:(mh + 1) * 512], pq,
                        bq_sb[:, jq:jq + 1])

            def k_proj(jq):
                for mh in range(2):
                    pk = ppr.tile([P, 512], F32, tag="ppr", name="pk")
                    for k in range(NCH):
                        nc.tensor.matmul(
                            pk,
                            lhsT=wk_sb[:, jq, k * P:(k + 1) * P],
                            rhs=ct_sb[:, k, mh * 512:(mh + 1) * 512],
                            start=(k == 0), stop=(k == NCH - 1),
                        )
                    nc.vector.tensor_scalar_add(
                        kt_sb[:, jq, mh * 512:(mh + 1) * 512], pk,
                        bk_sb[:, jq:jq + 1])

            # ---- V = c Wv + bv into the augmented per-head layout ----
            def v_phase(dh):
                for mm in range(NCH):
                    pv = ppr.tile([P, 512], F32, tag="ppr", name="pv")
                    for k in range(NCH):
                        nc.tensor.matmul(
                            pv,
                            lhsT=ct_sb[:, k, mm * P:(mm + 1) * P],
                            rhs=wv_sb[:, k, dh * 512:(dh + 1) * 512],
                            start=(k == 0), stop=(k == NCH - 1),
                        )
                    pvv = pv.rearrange("p (h e) -> p h e", e=HD)
                    bvv = bvb_sb[:, dh * 512:(dh + 1) * 512].rearrange(
                        "p (h e) -> p h e", e=HD)
                    nc.vector.tensor_add(
                        vaug4[:, mm, dh * 8:(dh + 1) * 8, 0:HD], pvv, bvv)

            # ---- attention blocks ----
            # Scores: ST[m,n] per head; head pair emitted as adjacent matmuls
            # at PE row bases 0/64 so both run concurrently (row tiling).
            def st_block(jh, nh, pool=None, ptag="ex"):
                nsl = slice(nh * 512, (nh + 1) * 512)
                # exAB[:, mm, h, :]: head h of pair jh, m-chunk mm
                exAB = (pool or expp).tile([P, NCH, 2, 512], BF16, tag=ptag,
                                           name="exAB")
                for mm in range(NCH):
                    # one 2-bank PSUM tile per m-chunk holding both heads:
                    # a single FD=1024 ACT frees A and B together (keeps the
                    # row-tiled pair adjacent) and bufs=2 double-buffers so
                    # the next chunk's matmuls overlap this chunk's exp.
                    ps = pst.tile([P, 2, 512], F32, tag="pst", name="ps")
                    msl = slice(mm * P, (mm + 1) * P)
                    nc.tensor.matmul(
                        ps[:, 0, :],
                        lhsT=kt_sb[0:HD, jh, msl],
                        rhs=qt_sb[0:HD, jh, nsl],
                        start=True, stop=True,
                    )
                    nc.tensor.matmul(
                        ps[:, 1, :],
                        lhsT=kt_sb[HD:P, jh, msl],
                        rhs=qt_sb[HD:P, jh, nsl],
                        start=True, stop=True,
                    )
                    nc.scalar.activation(exAB[:, mm, :, :], ps, EXP)
                return exAB, nsl

            # Softmax tails, batched 2 blocks at a time.  Block b parks
            # csA at partition 64b, csB at partition 64b+32 of a [97, 512]
            # tile (32-aligned bases as engines require); the tile is preset
            # to 1.0 so untouched rows stay finite through ln/exp.  One ln +
            # one exp ACT covers all 97 partitions (ACT cost is free-dim
            # based), then per block a K=33 selector matmul broadcasts local
            # row 0 to output rows 0-63 and local row 32 to rows 64-127.
            TB = 2
            selc = big.tile([97, P], BF16, tag="selc")
            nc.vector.memset(selc, 0.0)
            for pb in (0, 64):
                nc.vector.memset(selc[pb:pb + 1, 0:HD], 1.0)
                nc.vector.memset(selc[pb + 32:pb + 33, HD:P], 1.0)
            pend = []
            cur_cs = [None]

            def flush_tail():
                if not pend:
                    return
                csb = cur_cs[0]
                nb = len(pend)
                rlb = csp1.tile([97, 512], F32, tag="rl", name="rlb")
                rcb = csp1.tile([97, 512], BF16, tag="rc", name="rcb")
                nr = 64 * (nb - 1) + 33
                nc.scalar.activation(rlb[0:nr, :], csb[0:nr, :], LOG)
                nc.scalar.activation(rcb[0:nr, :], rlb[0:nr, :],
                                     EXP, scale=-1.0)
                for b, (otsA_, otsB_, jh_, nsl_) in enumerate(pend):
                    rbp = ppr.tile([P, 512], F32, tag="ppr", name="rbp")
                    nc.tensor.matmul(rbp, lhsT=selc[64 * b:64 * b + 33, :],
                                     rhs=rcb[64 * b:64 * b + 33, :],
                                     start=True, stop=True)
                    nc.vector.tensor_mul(
                        ot_sb[0:HD, jh_, nsl_], otsA_, rbp[0:HD, :])
                    nc.vector.tensor_mul(
                        ot_sb[HD:P, jh_, nsl_], otsB_, rbp[HD:P, :])
                pend.clear()
                cur_cs[0] = None

            def pv_block(jh, blk):
                exAB, nsl = blk
                poA = pot.tile([HD + 1, 512], F32, tag="pot", name="poA")
                for mm in range(NCH):
                    nc.tensor.matmul(
                        poA,
                        lhsT=vaug_sb[:, mm, (2 * jh) * HW:(2 * jh + 1) * HW],
                        rhs=exAB[:, mm, 0, :],
                        start=(mm == 0), stop=(mm == NCH - 1),
                    )
                poB = pot.tile([HD + 1, 512], F32, tag="pot", name="poB")
                for mm in range(NCH):
                    nc.tensor.matmul(
                        poB,
                        lhsT=vaug_sb[:, mm, (2 * jh + 1) * HW:(2 * jh + 2) * HW],
                        rhs=exAB[:, mm, 1, :],
                        start=(mm == 0), stop=(mm == NCH - 1),
                    )
                b = len(pend)
                if b == 0:
                    cur_cs[0] = csp.tile([97, 512], F32, tag="cs",
                                         name="csb")
                    nc.vector.memset(cur_cs[0], 1.0)
                csb = cur_cs[0]
                nc.vector.tensor_copy(csb[64 * b:64 * b + 1, :],
                                      poA[HD:HD + 1, :])
                nc.vector.tensor_copy(csb[64 * b + 32:64 * b + 33, :],
                                      poB[HD:HD + 1, :])
                otsA = otsp.tile([HD, 512], BF16, tag="ots", name="otsA")
                nc.vector.tensor_copy(otsA, poA[0:HD, :])
                otsB = otsp.tile([HD, 512], BF16, tag="ots", name="otsB")
                nc.vector.tensor_copy(otsB, poB[0:HD, :])
                pend.append((otsA, otsB, jh, nsl))
                if len(pend) == TB:
                    flush_tail()

            # ---- out = O Wo + bo for one 128-row, 512-col chunk ----
            def out_proj_half(nn, dh):
                pf = ppr.tile([P, 512], F32, tag="ppr", name="pf")
                for j in range(NCH):
                    nc.tensor.matmul(
                        pf,
                        lhsT=ot_sb[:, j, nn * P:(nn + 1) * P],
                        rhs=wo_sb[:, j, dh * 512:(dh + 1) * 512],
                        start=(j == 0), stop=(j == NCH - 1),
                    )
                of = outp.tile([P, 512], F32, tag="of", name="of")
                nc.vector.tensor_add(of, pf, bob_sb[:, dh * 512:(dh + 1) * 512])
                nc.sync.dma_start(
                    out=out_d[nn * P:(nn + 1) * P, dh * 512:(dh + 1) * 512],
                    in_=of)

            def out_proj(nn):
                out_proj_half(nn, 0)
                out_proj_half(nn, 1)

            # ---- the schedule ----
            blks = {}
            k_proj(0); q_proj(0)
            blks[(0, 0)] = st_block(0, 0)
            k_proj(1); q_proj(1)
            blks[(1, 0)] = st_block(1, 0)
            v_phase(0)
            pv_block(0, blks.pop((0, 0)))
            k_proj(2); q_proj(2)
            blks[(2, 0)] = st_block(2, 0)
            v_phase(1)
            pv_block(1, blks.pop((1, 0)))
            for j in range(3, NCH):
                k_proj(j); q_proj(j)
                blks[(j, 0)] = st_block(j, 0)
                pv_block(j - 1, blks.pop((j - 1, 0)))
            blks[(0, 1)] = st_block(0, 1)
            pv_block(7, blks.pop((7, 0)))
            # n-half 0's ot is complete after the flush inside pv_block(7,·);
            # spread out_proj(0..3) in half-units through the (ACT-bound)
            # second pass so TensorE has fill work in every block.
            units = [(nn, dh) for nn in range(4) for dh in range(2)]
            for j in range(1, NCH):
                blks[(j, 1)] = st_block(j, 1)
                pv_block(j - 1, blks.pop((j - 1, 1)))
                out_proj_half(*units.pop(0))
                if j >= 7:
                    out_proj_half(*units.pop(0))
            pv_block(7, blks.pop((7, 1)))
            flush_tail()
            # Warm-keeper: the last softmax tail leaves TensorE idle just
            # long enough for the HAM clock gate to re-throttle, making the
            # final out-projections run at half clock.  A burst of dummy
            # matmuls (no consumer) bridges the gap.
            warm = pst.tile([P, 2, 512], F32, tag="pst", name="warm")
            for w in range(12):
                nc.tensor.matmul(warm[:, w % 2, :],
                                 lhsT=ot_sb[:, 0, 0:P],
                                 rhs=ot_sb[:, 1, 0:512],
                                 start=True, stop=True)
            for nn in range(4, NCH):
                out_proj(nn)

    nc.compile()
    return nc


_STATE: dict = {}
LAST_EXEC_NS = None
LAST_PROFILE = None


def _prep_in_maps(x, context, Wq, bq, Wk, bk, Wv, bv, Wo, bo):
    def wpack_k(w, scale=1.0):
        # k-chunk major: [k, P, DIM]
        return (np.asarray(w, np.float32) * scale).astype(NPBF16).reshape(
            NCH, P, DIM)

    def wpack_jq(w, scale=1.0):
        # per-output-block: [jq, P(k-part), k*128]; w is [in, out]
        a = (np.asarray(w, np.float32) * scale).astype(NPBF16)
        a = a.reshape(NCH, P, NCH, P).transpose(2, 1, 0, 3)  # [jq, p, k, c]
        return np.ascontiguousarray(a.reshape(NCH, P, DIM))

    wq_r = wpack_jq(Wq, SCALE)
    wk_r = wpack_jq(Wk)
    wv_r = wpack_k(Wv)
    wo_r = wpack_k(Wo)
    bq_r = np.ascontiguousarray(
        (np.asarray(bq, np.float32) * SCALE).reshape(NCH, P).T)
    bk_r = np.ascontiguousarray(np.asarray(bk, np.float32).reshape(NCH, P).T)
    bv_r = np.asarray(bv, np.float32)
    bo_r = np.asarray(bo, np.float32)

    in_maps = []
    for c in range(B):
        xt_c = np.ascontiguousarray(np.asarray(x[c], np.float32).T).astype(
            NPBF16).reshape(NCH, P, SEQ)
        ct_c = np.ascontiguousarray(np.asarray(context[c], np.float32).T).astype(
            NPBF16).reshape(NCH, P, SEQ)
        in_maps.append({
            "xt": xt_c, "ct": ct_c,
            "wq": wq_r, "wk": wk_r, "wv": wv_r, "wo": wo_r,
            "bq": bq_r, "bk": bk_r, "bv": bv_r, "bo": bo_r,
        })
    return in_maps


def kernel(x, context, Wq, bq, Wk, bk, Wv, bv, Wo, bo):
    global LAST_EXEC_NS, LAST_PROFILE
    from concourse.bass_utils import run_bass_kernel_spmd

    if "nc" not in _STATE:
        _STATE["nc"] = build_nc()
    nc = _STATE["nc"]

    in_maps = _prep_in_maps(x, context, Wq, bq, Wk, bk, Wv, bv, Wo, bo)
    trace = bool(int(os.environ.get("KERNEL_TRACE", "0")))
    kw = {}
    tmpdir = os.environ.get("KERNEL_TMPDIR")
    if tmpdir:
        os.makedirs(tmpdir, exist_ok=True)
        kw["tmpdir"] = tmpdir
    res = run_bass_kernel_spmd(nc, in_maps, list(range(B)), trace=trace, **kw)
    LAST_EXEC_NS = res.exec_time_ns
    LAST_PROFILE = res.profile_json
    out = np.stack([res.results[c]["out"] for c in range(B)], axis=0)
    return out.astype(np.float32)
